# revision 17
# baseline (speedup 1.0000x reference)
"""Trainium2 Bass kernel for nn_BalancedTreeCell (binary-tree GNN message passing).

Math per batch row (independent per row -> pure data parallel over N=16 rows,
2 rows per NeuronCore on 8 cores):

  state = LN(input @ w_word + b_word)                       [S, D]
  repeat log2(S) times:
     l, r    = state[0::2], state[1::2]
     h       = gelu([l r] @ w1 + b1)                        [S/2, H]
     c       = h @ w2 + b2   -> f1,f2,i = sigmoid(c[:3D]), parent = c[3D:]
     state   = LN(f1*l + f2*r + i*parent)                   [S/2, D]
  out = state[0]                                            [D]

Key implementation choices (v2):
  - Feature-major state ([D partitions, tokens]) in SBUF for the whole tree;
    matmuls contract over features (partitions); even/odd token selection is
    a stride-2 free-dim view.  One unified cell path for every level.
  - bf16 for matmul operands (state / h / gates / parent / weights): same
    1 cyc/row PE rate as f32r at moving>=256, 4x faster below 256 (tail
    levels), enables FWL weight loads.  The pre-LN combine z, LN stats and
    per-token rows stay f32 for accuracy (measured 8.8e-3 max-rel).
  - sigmoid(u) = (1+tanh(u/2))/2 and LN scale-invariance:
        LN(f1*l+f2*r+i*p) = LN((l+r+p) + t1*l + t2*r + t3*p),
    with tj = tanh((cj+b2j)/2).  gelu/tanh/square/identity/copy all live in
    ACT function-table set 10, so the per-group table reloads (1.28us each)
    vanish; only the LN sqrt (set 3) switches, and it is batched.
  - Phase-pipelined scheduler: per unit M (matmuls+acts+combine) -> S (stats
    + rows chain) -> Q (batched sqrt) -> A (broadcast + apply), with S one
    unit behind M and A one unit behind Q; cross-level emission is
    demand-driven so level boundaries don't drain the pipe.  Small levels
    split into 2 units (one per batch row) so the two rows' serial chains
    overlap.
input_mask is all-ones per the problem spec, so the mask recursion is the
identity and is skipped.
"""

import numpy as np

import concourse.bass as bass
import concourse.bacc as bacc
import concourse.tile as tile
import concourse.mybir as mybir
from concourse.bass_utils import run_bass_kernel_spmd

F32 = mybir.dt.float32
F32R = mybir.dt.float32r
BF16 = mybir.dt.bfloat16
AF = mybir.ActivationFunctionType
ALU = mybir.AluOpType

P = 128
N_TOT = 16
S_FULL = 4096
D = 256
H = 1024
N_CORES = 8
R = N_TOT // N_CORES          # rows per core
DPT = D // P                  # 2 feature partition-tiles
EPS = 1e-5
G = 512                       # token group (PSUM bank = 512 fp32)
SPLIT = 32                    # row-split levels while To >= SPLIT


def _build(S=S_FULL, iters=1, max_levels=None):
    nc = bacc.Bacc("TRN2", target_bir_lowering=False, debug=False)
    T0 = R * S                       # flat tokens entering the tree

    # ---- DRAM I/O (per core) ----
    x_d = nc.dram_tensor("x", [T0, D], F32R, kind="ExternalInput").ap()
    w1_d = nc.dram_tensor("w1t", [P, 4, H], BF16, kind="ExternalInput").ap()
    w2_d = nc.dram_tensor("w2t", [P, 8, H], BF16, kind="ExternalInput").ap()
    ww_d = nc.dram_tensor("wwt", [P, 2, D], F32R, kind="ExternalInput").ap()
    b1c_d = nc.dram_tensor("b1c", [P, 8], F32, kind="ExternalInput").ap()
    b2c_d = nc.dram_tensor("b2c", [P, 8], F32, kind="ExternalInput").ap()
    b2h_d = nc.dram_tensor("b2h", [P, 8], F32, kind="ExternalInput").ap()
    bw_d = nc.dram_tensor("bwc", [P, 2], F32, kind="ExternalInput").ap()
    b1r_d = nc.dram_tensor("b1r", [8, P], BF16, kind="ExternalInput").ap()
    b2r_d = nc.dram_tensor("b2r", [8, P], BF16, kind="ExternalInput").ap()
    lnr_d = nc.dram_tensor("lnrow", [4, D], F32R, kind="ExternalInput").ap()
    lnb_d = nc.dram_tensor("lnbcol", [P, 2, 2], F32, kind="ExternalInput").ap()
    ones_d = nc.dram_tensor("onescol", [P, 1], F32R, kind="ExternalInput").ap()
    onesr_d = nc.dram_tensor("onesrow", [1, G], BF16, kind="ExternalInput").ap()
    eye_d = nc.dram_tensor("eye", [P, P], F32R, kind="ExternalInput").ap()
    eyeb_d = nc.dram_tensor("eyeb", [P, P], BF16, kind="ExternalInput").ap()
    out_d = nc.dram_tensor("out", [R, D], F32, kind="ExternalOutput").ap()

    with tile.TileContext(nc) as tc:
        cst = tc.alloc_tile_pool(name="cst", bufs=1)
        stp = tc.alloc_tile_pool(name="stp", bufs=1)
        ring_p = tc.alloc_tile_pool(name="ring", bufs=2)
        sb = tc.alloc_tile_pool(name="sb", bufs=2)
        sb3 = tc.alloc_tile_pool(name="sb3", bufs=3)
        hsb = tc.alloc_tile_pool(name="hsb", bufs=2)
        rows = tc.alloc_tile_pool(name="rows", bufs=4)
        ps = tc.alloc_tile_pool(name="ps", bufs=2, space="PSUM")
        psb = tc.alloc_tile_pool(name="psb", bufs=2, space="PSUM")

        # ---- constants ----
        w1s = cst.tile([P, 4, H], BF16)
        nc.sync.dma_start(out=w1s, in_=w1_d)
        w2s = cst.tile([P, 8, H], BF16)
        nc.sync.dma_start(out=w2s, in_=w2_d)
        wws = cst.tile([P, 2, D], F32R)
        nc.sync.dma_start(out=wws, in_=ww_d)
        b1cs = cst.tile([P, 8], F32)
        nc.sync.dma_start(out=b1cs, in_=b1c_d)
        b2cs = cst.tile([P, 8], F32)
        nc.sync.dma_start(out=b2cs, in_=b2c_d)
        b2hs = cst.tile([P, 8], F32)
        nc.sync.dma_start(out=b2hs, in_=b2h_d)
        bws = cst.tile([P, 2], F32)
        nc.sync.dma_start(out=bws, in_=bw_d)
        b1rs = []
        b2rs = []
        for m in range(8):
            t1 = cst.tile([1, P], BF16, name=f"b1r{m}")
            nc.sync.dma_start(out=t1, in_=b1r_d[m:m + 1, :])
            b1rs.append(t1)
            t2 = cst.tile([1, P], BF16, name=f"b2r{m}")
            nc.sync.dma_start(out=t2, in_=b2r_d[m:m + 1, :])
            b2rs.append(t2)
        lnrs = []
        for i in range(4):
            lt = cst.tile([1, D], F32R, name=f"lnr{i}")
            nc.sync.dma_start(out=lt, in_=lnr_d[i:i + 1, :])
            lnrs.append(lt)
        lnbs = cst.tile([P, 2, 2], F32)
        nc.sync.dma_start(out=lnbs, in_=lnb_d)
        oness = cst.tile([P, 1], F32R)
        nc.sync.dma_start(out=oness, in_=ones_d)
        onesr = cst.tile([1, G], BF16)
        nc.sync.dma_start(out=onesr, in_=onesr_d)
        eyes = cst.tile([P, P], F32R)
        nc.sync.dma_start(out=eyes, in_=eye_d)
        eyebs = cst.tile([P, P], BF16)
        nc.sync.dma_start(out=eyebs, in_=eyeb_d)

        # ---- persistent state buffers (feature-major [P, DPT, tokens]) ----
        TA = T0 // 2
        sA = stp.tile([P, DPT, TA], BF16, tag="sA", name="sA")
        sB = stp.tile([P, DPT, max(TA // 2, 1)], BF16, tag="sB", name="sB")

        def body():
            # =========================================================
            # unit table
            # =========================================================
            TR = min(1024, T0)            # ring tile tokens
            n_ring = T0 // TR
            nsub = max(TR // 512, 1)
            sub_t = min(512, TR)
            n_levels = int(np.log2(S)) if max_levels is None else max_levels

            units = []                    # dicts; index = unit id

            # stage0 subs: kind 's0', ring group rg, sub index
            # L1 groups:   kind 'l1', reads ring rg
            # level>=2:    kind 'lv'
            for rg in range(n_ring):
                for subi in range(nsub):
                    units.append(dict(kind="s0", rg=rg, sub=subi, Tg=sub_t,
                                      deps=[]))

            s0_uid = {}                   # (rg, sub) -> uid
            for i, u in enumerate(units):
                s0_uid[(u["rg"], u["sub"])] = i

            Tg1 = TR // 2
            l1_uids = []
            for g in range(n_ring):
                deps = [s0_uid[(g, s_)] for s_ in range(nsub)]
                units.append(dict(kind="l1", rg=g, Tg=Tg1,
                                  base_out=g * Tg1, deps=deps))
                l1_uids.append(len(units) - 1)

            # interleave stage0 + l1 emission order: s0 one ring ahead
            order = []
            for g in range(n_ring + 1):
                if g < n_ring:
                    for s_ in range(nsub):
                        order.append(s0_uid[(g, s_)])
                if g >= 1:
                    order.append(l1_uids[g - 1])

            # deeper levels
            lev_uids = [None, l1_uids]    # lev_uids[l] = unit ids of level l
            Tin = T0 // 2
            lev = 1
            while Tin > 1 and lev < n_levels:
                To = Tin // 2
                if To >= 2 * G:
                    ng = To // G
                elif To >= SPLIT:
                    ng = 2
                else:
                    ng = 1
                Tg = To // ng
                prev_ids = lev_uids[lev]
                prev_Tg = units[prev_ids[0]]["Tg"]
                ids = []
                for g in range(ng):
                    in_lo, in_hi = 2 * g * Tg, 2 * (g + 1) * Tg - 1
                    deps = [prev_ids[in_lo // prev_Tg + k]
                            for k in range(in_hi // prev_Tg -
                                           in_lo // prev_Tg + 1)]
                    units.append(dict(kind="lv", lev=lev + 1, Tg=Tg,
                                      base_out=g * Tg, base_in=2 * g * Tg,
                                      deps=deps))
                    ids.append(len(units) - 1)
                    order.append(len(units) - 1)
                lev_uids.append(ids)
                Tin = To
                lev += 1

            # Q batches: pair consecutive same-level units when the level
            # has >= 4 units (L1/L2 + stage0); else singleton.
            batch_of = {}
            batches = {}                  # bid -> [uids]
            bid_ctr = [0]

            def new_batch(uids):
                b = bid_ctr[0]
                bid_ctr[0] += 1
                batches[b] = uids
                for x in uids:
                    batch_of[x] = b

            for l, ids in enumerate(lev_uids):
                if ids is None:
                    continue
                if len(ids) >= 4:
                    for j in range(0, len(ids) - 1, 2):
                        new_batch([ids[j], ids[j + 1]])
                    if len(ids) % 2:
                        new_batch([ids[-1]])
                else:
                    for x in ids:
                        new_batch([x])
            s0_ids = [i for i, u in enumerate(units) if u["kind"] == "s0"]
            for j in range(0, len(s0_ids) - 1, 2):
                new_batch([s0_ids[j], s0_ids[j + 1]])
            if len(s0_ids) % 2:
                new_batch([s0_ids[-1]])

            # =========================================================
            # per-unit tile state (filled in by M/S phases)
            # =========================================================
            ustate = [dict() for _ in units]

            def state_buf_for(lv):
                # level l output buffer: odd levels -> sA, even -> sB
                return sA if lv % 2 == 1 else sB

            def in_aps(u):
                """xk k-tile APs [l0,l1,r0,r1] + lr refs for the combine."""
                if u["kind"] == "l1":
                    rt = ustate[s0_uid[(u["rg"], 0)]]["ring"]
                    base = 0
                    src = rt
                else:
                    src = state_buf_for(u["lev"] - 1)
                    base = u["base_in"]
                Tg = u["Tg"]
                xk = [src[:, 0, base:base + 2 * Tg:2],
                      src[:, 1, base:base + 2 * Tg:2],
                      src[:, 0, base + 1:base + 2 * Tg:2],
                      src[:, 1, base + 1:base + 2 * Tg:2]]
                return xk

            def dst_for(u):
                if u["kind"] == "l1":
                    return sA
                return sA if u["lev"] % 2 == 1 else sB

            def out_aps(u):
                if u["kind"] == "s0":
                    rt = ustate[s0_uid[(u["rg"], 0)]]["ring"]
                    lo = u["sub"] * u["Tg"]
                    return [rt[:, pt, lo:lo + u["Tg"]] for pt in range(DPT)]
                dst = dst_for(u)
                lo = u["base_out"]
                return [dst[:, pt, lo:lo + u["Tg"]] for pt in range(DPT)]

            # =========================================================
            # M phase
            # =========================================================
            def m_cell(u):
                Tg = u["Tg"]
                xk = in_aps(u)
                mpb = max(1, min(8, G // Tg))      # m-tiles per PSUM bank
                small = mpb > 1
                # ---- mm1 + gelu -> h ----
                h = hsb.tile([P, 8, G], BF16, tag="h", name="h")[:, :, :Tg]
                for b0 in range(0, 8, mpb):
                    nb = min(mpb, 8 - b0)
                    ph = ps.tile([P, mpb, G // mpb], F32, tag="mm",
                                 name="ph")[:, :nb, :Tg]
                    for mi in range(nb):
                        m = b0 + mi
                        for k in range(4):
                            nc.tensor.matmul(
                                ph[:, mi, :], lhsT=w1s[:, k, m * P:(m + 1) * P],
                                rhs=xk[k], start=(k == 0),
                                stop=(k == 3 and not small))
                        if small:
                            nc.tensor.matmul(
                                ph[:, mi, :], lhsT=b1rs[m],
                                rhs=onesr[:, :Tg], start=False, stop=True)
                    if small:
                        nc.scalar.activation(out=h[:, b0:b0 + nb, :], in_=ph,
                                             func=AF.Gelu)
                    else:
                        nc.scalar.activation(out=h[:, b0, :], in_=ph[:, 0, :],
                                             func=AF.Gelu,
                                             bias=b1cs[:, b0:b0 + 1])
                # ---- mm2 -> par (ident) + tanh gates ----
                par = sb.tile([P, 2, G], BF16, tag="par", name="par")[:, :, :Tg]
                gt = sb.tile([P, 6, G], BF16, tag="gt", name="gt")[:, :, :Tg]
                z = sb3.tile([P, DPT, G], F32R, tag="z", name="z")[:, :, :Tg]
                sacc = sb.tile([P, DPT, G], F32, tag="sacc",
                               name="sacc")[:, :, :Tg]
                m2_order = [6, 7, 0, 1, 2, 3, 4, 5]   # parent first
                bi = 0
                while bi < 8:
                    nb = min(mpb, 8 - bi)
                    ms_arr = m2_order[bi:bi + nb]
                    pc = ps.tile([P, mpb, G // mpb], F32, tag="mm",
                                 name="pc")[:, :nb, :Tg]
                    for mi, m2 in enumerate(ms_arr):
                        for k in range(8):
                            nc.tensor.matmul(
                                pc[:, mi, :],
                                lhsT=w2s[:, k, m2 * P:(m2 + 1) * P],
                                rhs=h[:, k, :], start=(k == 0),
                                stop=(k == 7 and not small))
                        if small:
                            nc.tensor.matmul(
                                pc[:, mi, :], lhsT=b2rs[m2],
                                rhs=onesr[:, :Tg], start=False, stop=True)
                    bi += nb
                    # consume this bank with ACT immediately
                    if small:
                        # contiguous runs of parent (6,7) vs gates (0..5)
                        st_ = 0
                        while st_ < len(ms_arr):
                            en = st_
                            is_par = ms_arr[st_] >= 6
                            while en + 1 < len(ms_arr) and \
                                    (ms_arr[en + 1] >= 6) == is_par:
                                en += 1
                            n_ = en - st_ + 1
                            if is_par:
                                nc.scalar.activation(
                                    out=par[:, ms_arr[st_] - 6:
                                            ms_arr[st_] - 6 + n_, :],
                                    in_=pc[:, st_:en + 1, :],
                                    func=AF.Identity)
                            else:
                                nc.scalar.activation(
                                    out=gt[:, ms_arr[st_]:
                                           ms_arr[st_] + n_, :],
                                    in_=pc[:, st_:en + 1, :],
                                    func=AF.Tanh, scale=0.5)
                            st_ = en + 1
                    else:
                        m2 = ms_arr[0]
                        if m2 >= 6:
                            nc.scalar.activation(
                                out=par[:, m2 - 6, :], in_=pc[:, 0, :],
                                func=AF.Identity, bias=b2cs[:, m2:m2 + 1])
                        else:
                            nc.scalar.activation(
                                out=gt[:, m2, :], in_=pc[:, 0, :],
                                func=AF.Tanh, scale=0.5,
                                bias=b2hs[:, m2:m2 + 1])
                # ---- combine: z = (l+r+par) + t1*l + t2*r + t3*par ----
                lr = xk
                m2p = sb.tile([P, DPT, G], F32, tag="m2p",
                              name="m2p")[:, :, :Tg]
                m3p = sb.tile([P, DPT, G], F32, tag="m3p",
                              name="m3p")[:, :, :Tg]
                for pt in range(DPT):
                    # s-chain on DVE (inputs available early)
                    nc.vector.tensor_add(sacc[:, pt, :], lr[pt], lr[2 + pt])
                    nc.vector.tensor_add(sacc[:, pt, :], sacc[:, pt, :],
                                         par[:, pt, :])
                    # gate products: t1*l on DVE, t2*r / t3*par on Pool
                    nc.gpsimd.tensor_mul(m2p[:, pt, :], gt[:, 2 + pt, :],
                                         lr[2 + pt])
                    nc.gpsimd.tensor_mul(m3p[:, pt, :], gt[:, 4 + pt, :],
                                         par[:, pt, :])
                    nc.vector.tensor_mul(z[:, pt, :], gt[:, pt, :], lr[pt])
                    nc.vector.tensor_add(z[:, pt, :], z[:, pt, :],
                                         sacc[:, pt, :])
                    nc.vector.tensor_add(z[:, pt, :], z[:, pt, :],
                                         m2p[:, pt, :])
                    nc.vector.tensor_add(z[:, pt, :], z[:, pt, :],
                                         m3p[:, pt, :])
                ustate[u["_id"]]["z"] = z

            def m_stage0(u):
                Tg = u["Tg"]
                rg, subi = u["rg"], u["sub"]
                if subi == 0:
                    rt = ring_p.tile([P, DPT, TR], BF16, tag="ring",
                                     name="ring")
                    ustate[u["_id"]]["ring"] = rt
                    for s_ in range(1, nsub):
                        ustate[s0_uid[(rg, s_)]]["ring"] = rt
                gi = rg * nsub + subi
                xr = x_d.rearrange("(a s p) d -> a p s d", p=P,
                                   s=max(sub_t // P, 1))
                itm = sb3.tile([P, max(sub_t // P, 1), D], F32R, tag="itm",
                               name="itm", bufs=2)
                nc.sync.dma_start(out=itm, in_=xr[gi])
                x0 = sb.tile([P, DPT, 512], F32R, tag="x0",
                             name="x0", bufs=1)[:, :, :Tg]
                for pt in range(DPT):
                    xtp = ps.tile([P, 512], F32R, tag="bc",
                                  name="xtp", bufs=2)[:, :Tg]
                    for s_ in range(max(sub_t // P, 1)):
                        nc.tensor.transpose(
                            xtp[:, s_ * P:(s_ + 1) * P],
                            itm[:, s_, pt * P:(pt + 1) * P], eyes)
                    nc.scalar.copy(out=x0[:, pt, :], in_=xtp)
                z = sb3.tile([P, DPT, G], F32R, tag="z", name="z")[:, :, :Tg]
                for pt in range(DPT):
                    pw = ps.tile([P, 1, G], F32, tag="mm",
                                 name="pw")[:, 0, :Tg]
                    for k in range(DPT):
                        nc.tensor.matmul(pw, lhsT=wws[:, k, pt * P:(pt + 1) * P],
                                         rhs=x0[:, k, :],
                                         start=(k == 0), stop=(k == DPT - 1))
                    nc.scalar.activation(out=z[:, pt, :], in_=pw,
                                         func=AF.Identity,
                                         bias=bws[:, pt:pt + 1])
                ustate[u["_id"]]["z"] = z

            # =========================================================
            # S phase: stats matmuls + rows chain through rvar
            # =========================================================
            def s_phase(u):
                Tg = u["Tg"]
                z = ustate[u["_id"]]["z"]
                zsq = sb.tile([P, DPT, G], F32R, tag="zsq",
                              name="zsq")[:, :, :Tg]
                nc.scalar.activation(out=zsq, in_=z, func=AF.Square)
                st = psb.tile([1, G], F32, tag="st", name="st")[:, :Tg]
                sq = psb.tile([1, G], F32, tag="sq", name="sq")[:, :Tg]
                for pt in range(DPT):
                    nc.tensor.matmul(st, lhsT=oness, rhs=z[:, pt, :],
                                     start=(pt == 0), stop=(pt == DPT - 1))
                for pt in range(DPT):
                    nc.tensor.matmul(sq, lhsT=oness, rhs=zsq[:, pt, :],
                                     start=(pt == 0), stop=(pt == DPT - 1))
                mu = rows.tile([1, G], F32, tag="mu", name="mu")[:, :Tg]
                nc.vector.tensor_scalar_mul(mu, st, 1.0 / D)
                ex2 = rows.tile([1, G], F32, tag="ex2", name="ex2",
                                bufs=2)[:, :Tg]
                nc.vector.tensor_scalar_mul(ex2, sq, 1.0 / D)
                mq = rows.tile([1, G], F32, tag="mq", name="mq",
                               bufs=2)[:, :Tg]
                nc.vector.tensor_mul(mq, mu, mu)
                b = batch_of[u["_id"]]
                slot = batches[b].index(u["_id"])
                if "vb" not in ustate[batches[b][0]]:
                    vb = rows.tile([1, 2, G], F32, tag="vb", name="vb",
                                   bufs=2)
                    rsb = rows.tile([1, 2, G], F32R, tag="rsb", name="rsb",
                                    bufs=2)
                    for x in batches[b]:
                        ustate[x]["vb"] = vb
                        ustate[x]["rsb"] = rsb
                        ustate[x]["slot"] = batches[b].index(x)
                vb = ustate[u["_id"]]["vb"]
                nc.vector.scalar_tensor_tensor(
                    out=ex2, in0=ex2, scalar=EPS,
                    in1=mq, op0=ALU.add, op1=ALU.subtract)
                nc.vector.reciprocal(out=vb[:, slot, :Tg], in_=ex2)
                ustate[u["_id"]]["mu"] = mu

            # =========================================================
            # Q phase: batched sqrt (rsig = sqrt(1/var))
            # =========================================================
            def q_phase(bid):
                us = batches[bid]
                vb = ustate[us[0]]["vb"]
                rsb = ustate[us[0]]["rsb"]
                Tg = units[us[0]]["Tg"]
                if len(us) == 2:
                    nc.scalar.activation(out=rsb[:, :, :Tg],
                                         in_=vb[:, :, :Tg], func=AF.Sqrt)
                else:
                    nc.scalar.activation(out=rsb[:, 0, :Tg],
                                         in_=vb[:, 0, :Tg], func=AF.Sqrt)

            # =========================================================
            # A phase: ms, broadcast matmuls, apply -> state (bf16)
            # =========================================================
            def a_phase(u):
                Tg = u["Tg"]
                z = ustate[u["_id"]]["z"]
                mu = ustate[u["_id"]]["mu"]
                rsb = ustate[u["_id"]]["rsb"]
                slot = ustate[u["_id"]]["slot"]
                ln = 0 if u["kind"] == "s0" else 1
                rsig = rsb[:, slot, :Tg]
                ms = rows.tile([1, G], F32R, tag="ms", name="ms",
                               bufs=2)[:, :Tg]
                nc.vector.tensor_mul(ms, mu, rsig)
                oaps = out_aps(u)
                for pt in range(DPT):
                    b1p = ps.tile([P, G], F32, tag="bc", name="bc",
                                  bufs=2)[:, :Tg]
                    nc.tensor.matmul(b1p, lhsT=lnrs[2 * ln][:, pt * P:(pt + 1) * P],
                                     rhs=rsig, start=True, stop=True)
                    b2p = ps.tile([P, G], F32, tag="bc", name="bc",
                                  bufs=2)[:, :Tg]
                    nc.tensor.matmul(b2p, lhsT=lnrs[2 * ln + 1][:, pt * P:(pt + 1) * P],
                                     rhs=ms, start=True, stop=True)
                    t = sb.tile([P, G], F32, tag="tap", name="tap")[:, :Tg]
                    nc.vector.tensor_mul(t, z[:, pt, :], b1p)
                    nc.vector.scalar_tensor_tensor(
                        out=oaps[pt], in0=t, scalar=lnbs[:, ln, pt:pt + 1],
                        in1=b2p, op0=ALU.add, op1=ALU.add)

            # =========================================================
            # demand-driven scheduler
            # =========================================================
            PH_M, PH_S, PH_Q, PH_A = 0, 1, 2, 3
            phase = [None] * len(units)   # highest phase emitted
            q_done = set()
            q_tick = {}
            tick = [0]

            for i, u in enumerate(units):
                u["_id"] = i

            def emit_m(i):
                u = units[i]
                if u["kind"] == "s0":
                    m_stage0(u)
                else:
                    m_cell(u)
                phase[i] = PH_M

            def emit_s(i):
                if phase[i] is None:
                    emit_m(i)
                if phase[i] < PH_S:
                    s_phase(units[i])
                    phase[i] = PH_S

            def emit_q(bid):
                if bid in q_done:
                    return
                for x in batches[bid]:
                    emit_s(x)
                q_phase(bid)
                q_tick[bid] = tick[0]
                q_done.add(bid)
                for x in batches[bid]:
                    phase[x] = PH_Q

            def emit_a(i):
                if phase[i] is not None and phase[i] >= PH_A:
                    return
                emit_q(batch_of[i])
                a_phase(units[i])
                phase[i] = PH_A

            # main loop
            a_pending = []                # unit ids with Q done, A not
            for oi, i in enumerate(order):
                u = units[i]
                for d in u["deps"]:
                    emit_a(d)
                emit_m(i)
                tick[0] = oi
                # lagged S for the previously ordered unit
                if oi >= 1:
                    j = order[oi - 1]
                    if phase[j] < PH_S:
                        emit_s(j)
                        bid = batch_of[j]
                        if all(phase[x] is not None and phase[x] >= PH_S
                               for x in batches[bid]):
                            emit_q(bid)
                            a_pending.extend(batches[bid])
                # A for units whose Q is at least one tick old
                still = []
                for j in a_pending:
                    if phase[j] >= PH_A:
                        continue
                    if q_tick[batch_of[j]] < tick[0]:
                        emit_a(j)
                    else:
                        still.append(j)
                a_pending = still
            for i in range(len(units)):
                emit_a(i)

            # ---- emit output [R, D] (last level wrote 2 tokens) ----
            last_ids = lev_uids[-1]
            dst = dst_for(units[last_ids[0]])
            outt = sb.tile([R, D], F32, tag="outt", name="outt")
            for pt in range(DPT):
                otp = psb.tile([R, P], BF16, tag="st", name="otp")
                nc.tensor.transpose(otp, dst[:, pt, 0:R], eyebs)
                nc.vector.tensor_copy(out=outt[:, pt * P:(pt + 1) * P],
                                      in_=otp)
            nc.sync.dma_start(out=out_d, in_=outt)

        if iters == 1:
            body()
        else:
            with tc.For_i(0, iters, 1):
                body()

        for p_ in (psb, ps, rows, hsb, sb3, sb, ring_p, stp, cst):
            p_.release()

    nc.compile()
    return nc


def _prep_weights(w_word, b_word, w1, bias1, w2, bias2,
                  ln0_g, ln0_b, lnc_g, lnc_b):
    import ml_dtypes
    f = np.float32
    bf = ml_dtypes.bfloat16
    w1h = np.ascontiguousarray(
        w1.reshape(4, P, H).transpose(1, 0, 2), dtype=bf)
    w2h = np.ascontiguousarray(
        w2.reshape(8, P, H).transpose(1, 0, 2), dtype=bf)
    wwh = np.ascontiguousarray(w_word.reshape(2, P, D).transpose(1, 0, 2),
                               dtype=f)
    b1h = np.ascontiguousarray(bias1.reshape(8, P).T, dtype=f)
    b2h = np.ascontiguousarray(bias2.reshape(8, P).T, dtype=f)
    bwh = np.ascontiguousarray(b_word.reshape(2, P).T, dtype=f)
    lnrow = np.stack([ln0_g, -ln0_g, lnc_g, -lnc_g]).astype(f)    # [4, D]
    lnbcol = np.ascontiguousarray(
        np.stack([ln0_b, lnc_b]).reshape(2, 2, P).transpose(2, 0, 1), dtype=f)
    return dict(w1t=w1h, w2t=w2h, wwt=wwh, b1c=b1h, b2c=b2h,
                b2h=(b2h * 0.5).astype(f), bwc=bwh,
                b1r=np.ascontiguousarray(bias1.reshape(8, P), dtype=bf),
                b2r=np.ascontiguousarray(bias2.reshape(8, P), dtype=bf),
                lnrow=lnrow, lnbcol=lnbcol,
                onescol=np.ones((P, 1), f),
                onesrow=np.ones((1, G), bf),
                eye=np.eye(P, dtype=f),
                eyeb=np.eye(P, dtype=bf))


_NC_CACHE = {}


def _get_nc(S=S_FULL, iters=1, max_levels=None):
    key = (S, iters, max_levels)
    if key not in _NC_CACHE:
        _NC_CACHE[key] = _build(S, iters, max_levels)
    return _NC_CACHE[key]


def kernel(input, input_mask, w_word, b_word, w1, bias1, w2, bias2,
           ln0_g, ln0_b, lnc_g, lnc_b, _iters=1, _max_levels=None):
    inp = np.asarray(input, dtype=np.float32)
    shared = _prep_weights(
        np.asarray(w_word), np.asarray(b_word), np.asarray(w1),
        np.asarray(bias1), np.asarray(w2), np.asarray(bias2),
        np.asarray(ln0_g), np.asarray(ln0_b), np.asarray(lnc_g),
        np.asarray(lnc_b))
    S = inp.shape[1]
    nc = _get_nc(S, _iters, _max_levels)
    in_maps = []
    for c in range(N_CORES):
        m = dict(shared)
        m["x"] = np.ascontiguousarray(
            inp[c * R:(c + 1) * R].reshape(R * S, D))
        in_maps.append(m)
    res = run_bass_kernel_spmd(nc, in_maps, core_ids=list(range(N_CORES)))
    return np.concatenate([res.results[c]["out"] for c in range(N_CORES)],
                          axis=0)


# revision 24
# speedup vs baseline: 33.7951x; 33.7951x over previous
"""Trainium2 Bass kernel for nn_BalancedTreeCell (binary-tree GNN message passing).

Math per batch row (independent per row -> pure data parallel over N=16 rows,
2 rows per NeuronCore on 8 cores):

  state = LN(input @ w_word + b_word)                       [S, D]
  repeat log2(S) times:
     l, r    = state[0::2], state[1::2]
     h       = gelu([l r] @ w1 + b1)                        [S/2, H]
     c       = h @ w2 + b2   -> f1,f2,i = sigmoid(c[:3D]), parent = c[3D:]
     state   = LN(f1*l + f2*r + i*parent)                   [S/2, D]
  out = state[0]                                            [D]

Key implementation choices (v2):
  - Feature-major state ([D partitions, tokens]) in SBUF for the whole tree;
    matmuls contract over features (partitions); even/odd token selection is
    a stride-2 free-dim view.  One unified cell path for every level.
  - bf16 for matmul operands (state / h / gates / parent / weights): same
    1 cyc/row PE rate as f32r at moving>=256, 4x faster below 256 (tail
    levels), enables FWL weight loads.  The pre-LN combine z, LN stats and
    per-token rows stay f32 for accuracy (measured 8.8e-3 max-rel).
  - sigmoid(u) = (1+tanh(u/2))/2 and LN scale-invariance:
        LN(f1*l+f2*r+i*p) = LN((l+r+p) + t1*l + t2*r + t3*p),
    with tj = tanh((cj+b2j)/2).  gelu/tanh/square/identity/copy all live in
    ACT function-table set 10, so the per-group table reloads (1.28us each)
    vanish; only the LN sqrt (set 3) switches, and it is batched.
  - Phase-pipelined scheduler: per unit M (matmuls+acts+combine) -> S (stats
    + rows chain) -> Q (batched sqrt) -> A (broadcast + apply), with S one
    unit behind M and A one unit behind Q; cross-level emission is
    demand-driven so level boundaries don't drain the pipe.  Small levels
    split into 2 units (one per batch row) so the two rows' serial chains
    overlap.
input_mask is all-ones per the problem spec, so the mask recursion is the
identity and is skipped.
"""

import numpy as np

import concourse.bass as bass
import concourse.bacc as bacc
import concourse.tile as tile
import concourse.mybir as mybir
from concourse.bass_utils import run_bass_kernel_spmd

F32 = mybir.dt.float32
F32R = mybir.dt.float32r
BF16 = mybir.dt.bfloat16
AF = mybir.ActivationFunctionType
ALU = mybir.AluOpType

P = 128
N_TOT = 16
S_FULL = 4096
D = 256
H = 1024
N_CORES = 8
R = N_TOT // N_CORES          # rows per core
DPT = D // P                  # 2 feature partition-tiles
EPS = 1e-5
G = 512                       # token group (PSUM bank = 512 fp32)
SPLIT = 32                    # row-split levels while To >= SPLIT


def _build(S=S_FULL, iters=1, max_levels=None):
    nc = bacc.Bacc("TRN2", target_bir_lowering=False, debug=False)
    T0 = R * S                       # flat tokens entering the tree

    # ---- DRAM I/O (per core) ----
    x_d = nc.dram_tensor("x", [T0, D], F32R, kind="ExternalInput").ap()
    w1_d = nc.dram_tensor("w1t", [P, 4, H], BF16, kind="ExternalInput").ap()
    w2_d = nc.dram_tensor("w2t", [P, 8, H], BF16, kind="ExternalInput").ap()
    ww_d = nc.dram_tensor("wwt", [P, 2, D], F32R, kind="ExternalInput").ap()
    b1c_d = nc.dram_tensor("b1c", [P, 8], F32, kind="ExternalInput").ap()
    b2c_d = nc.dram_tensor("b2c", [P, 8], F32, kind="ExternalInput").ap()
    b2h_d = nc.dram_tensor("b2h", [P, 8], F32, kind="ExternalInput").ap()
    bw_d = nc.dram_tensor("bwc", [P, 2], F32, kind="ExternalInput").ap()
    b1r_d = nc.dram_tensor("b1r", [8, P], BF16, kind="ExternalInput").ap()
    b2r_d = nc.dram_tensor("b2r", [8, P], BF16, kind="ExternalInput").ap()
    lnr_d = nc.dram_tensor("lnrow", [4, D], F32R, kind="ExternalInput").ap()
    lnb_d = nc.dram_tensor("lnbcol", [P, 2, 2], F32, kind="ExternalInput").ap()
    ones_d = nc.dram_tensor("onescol", [P, 1], F32R, kind="ExternalInput").ap()
    onesr_d = nc.dram_tensor("onesrow", [1, G], BF16, kind="ExternalInput").ap()
    eye_d = nc.dram_tensor("eye", [P, P], F32R, kind="ExternalInput").ap()
    eyeb_d = nc.dram_tensor("eyeb", [P, P], BF16, kind="ExternalInput").ap()
    out_d = nc.dram_tensor("out", [R, D], F32, kind="ExternalOutput").ap()

    with tile.TileContext(nc) as tc:
        cst = tc.alloc_tile_pool(name="cst", bufs=1)
        stp = tc.alloc_tile_pool(name="stp", bufs=1)
        ring_p = tc.alloc_tile_pool(name="ring", bufs=2)
        sb = tc.alloc_tile_pool(name="sb", bufs=2)
        sb3 = tc.alloc_tile_pool(name="sb3", bufs=3)
        hsb = tc.alloc_tile_pool(name="hsb", bufs=2)
        rows = tc.alloc_tile_pool(name="rows", bufs=4)
        ps = tc.alloc_tile_pool(name="ps", bufs=2, space="PSUM")
        psb = tc.alloc_tile_pool(name="psb", bufs=2, space="PSUM")

        # ---- constants ----
        w1s = cst.tile([P, 4, H], BF16)
        nc.sync.dma_start(out=w1s, in_=w1_d)
        w2s = cst.tile([P, 8, H], BF16)
        nc.sync.dma_start(out=w2s, in_=w2_d)
        wws = cst.tile([P, 2, D], F32R)
        nc.sync.dma_start(out=wws, in_=ww_d)
        b1cs = cst.tile([P, 8], F32)
        nc.sync.dma_start(out=b1cs, in_=b1c_d)
        b2cs = cst.tile([P, 8], F32)
        nc.sync.dma_start(out=b2cs, in_=b2c_d)
        b2hs = cst.tile([P, 8], F32)
        nc.sync.dma_start(out=b2hs, in_=b2h_d)
        bws = cst.tile([P, 2], F32)
        nc.sync.dma_start(out=bws, in_=bw_d)
        b1rs = []
        b2rs = []
        for m in range(8):
            t1 = cst.tile([1, P], BF16, name=f"b1r{m}")
            nc.sync.dma_start(out=t1, in_=b1r_d[m:m + 1, :])
            b1rs.append(t1)
            t2 = cst.tile([1, P], BF16, name=f"b2r{m}")
            nc.sync.dma_start(out=t2, in_=b2r_d[m:m + 1, :])
            b2rs.append(t2)
        lnrs = []
        for i in range(4):
            lt = cst.tile([1, D], F32R, name=f"lnr{i}")
            nc.sync.dma_start(out=lt, in_=lnr_d[i:i + 1, :])
            lnrs.append(lt)
        lnbs = cst.tile([P, 2, 2], F32)
        nc.sync.dma_start(out=lnbs, in_=lnb_d)
        oness = cst.tile([P, 1], F32R)
        nc.sync.dma_start(out=oness, in_=ones_d)
        onesr = cst.tile([1, G], BF16)
        nc.sync.dma_start(out=onesr, in_=onesr_d)
        eyes = cst.tile([P, P], F32R)
        nc.sync.dma_start(out=eyes, in_=eye_d)
        eyebs = cst.tile([P, P], BF16)
        nc.sync.dma_start(out=eyebs, in_=eyeb_d)

        # ---- persistent state buffers (feature-major [P, DPT, tokens]) ----
        TA = T0 // 2
        sA = stp.tile([P, DPT, TA], BF16, tag="sA", name="sA")
        sB = stp.tile([P, DPT, max(TA // 2, 1)], BF16, tag="sB", name="sB")

        def body():
            # =========================================================
            # unit table
            # =========================================================
            TR = min(1024, T0)            # ring tile tokens
            n_ring = T0 // TR
            nsub = max(TR // 512, 1)
            sub_t = min(512, TR)
            n_levels = int(np.log2(S)) if max_levels is None else max_levels

            units = []                    # dicts; index = unit id

            # stage0 subs: kind 's0', ring group rg, sub index
            # L1 groups:   kind 'l1', reads ring rg
            # level>=2:    kind 'lv'
            for rg in range(n_ring):
                for subi in range(nsub):
                    units.append(dict(kind="s0", rg=rg, sub=subi, Tg=sub_t,
                                      deps=[]))

            s0_uid = {}                   # (rg, sub) -> uid
            for i, u in enumerate(units):
                s0_uid[(u["rg"], u["sub"])] = i

            Tg1 = TR // 2
            l1_uids = []
            for g in range(n_ring):
                deps = [s0_uid[(g, s_)] for s_ in range(nsub)]
                units.append(dict(kind="l1", rg=g, Tg=Tg1,
                                  base_out=g * Tg1, deps=deps))
                l1_uids.append(len(units) - 1)

            # interleave stage0 + l1 emission order: s0 one ring ahead
            order = []
            for g in range(n_ring + 1):
                if g < n_ring:
                    for s_ in range(nsub):
                        order.append(s0_uid[(g, s_)])
                if g >= 1:
                    order.append(l1_uids[g - 1])

            # deeper levels
            lev_uids = [None, l1_uids]    # lev_uids[l] = unit ids of level l
            Tin = T0 // 2
            lev = 1
            while Tin > 1 and lev < n_levels:
                To = Tin // 2
                if To >= 2 * G:
                    ng = To // G
                elif To >= SPLIT:
                    ng = 2
                else:
                    ng = 1
                Tg = To // ng
                prev_ids = lev_uids[lev]
                prev_Tg = units[prev_ids[0]]["Tg"]
                ids = []
                for g in range(ng):
                    in_lo, in_hi = 2 * g * Tg, 2 * (g + 1) * Tg - 1
                    deps = [prev_ids[in_lo // prev_Tg + k]
                            for k in range(in_hi // prev_Tg -
                                           in_lo // prev_Tg + 1)]
                    units.append(dict(kind="lv", lev=lev + 1, Tg=Tg,
                                      base_out=g * Tg, base_in=2 * g * Tg,
                                      deps=deps))
                    ids.append(len(units) - 1)
                    order.append(len(units) - 1)
                lev_uids.append(ids)
                Tin = To
                lev += 1

            # Q batches: pair consecutive same-level units when the level
            # has >= 4 units (L1/L2 + stage0); else singleton.
            batch_of = {}
            batches = {}                  # bid -> [uids]
            bid_ctr = [0]

            def new_batch(uids):
                b = bid_ctr[0]
                bid_ctr[0] += 1
                batches[b] = uids
                for x in uids:
                    batch_of[x] = b

            m_fused = set()               # bids whose M phases are fused
            for l, ids in enumerate(lev_uids):
                if ids is None:
                    continue
                if len(ids) >= 2 and units[ids[0]]["Tg"] >= 512:
                    for j in range(0, len(ids) - 1, 2):
                        new_batch([ids[j], ids[j + 1]])
                        m_fused.add(bid_ctr[0] - 1)
                    if len(ids) % 2:
                        new_batch([ids[-1]])
                else:
                    for x in ids:
                        new_batch([x])
            s0_ids = [i for i, u in enumerate(units) if u["kind"] == "s0"]
            for j in range(0, len(s0_ids) - 1, 2):
                new_batch([s0_ids[j], s0_ids[j + 1]])
            if len(s0_ids) % 2:
                new_batch([s0_ids[-1]])

            # =========================================================
            # per-unit tile state (filled in by M/S phases)
            # =========================================================
            ustate = [dict() for _ in units]

            def state_buf_for(lv):
                # level l output buffer: odd levels -> sA, even -> sB
                return sA if lv % 2 == 1 else sB

            def in_aps(u):
                """xk k-tile APs [l0,l1,r0,r1] + lr refs for the combine."""
                if u["kind"] == "l1":
                    rt = ustate[s0_uid[(u["rg"], 0)]]["ring"]
                    base = 0
                    src = rt
                else:
                    src = state_buf_for(u["lev"] - 1)
                    base = u["base_in"]
                Tg = u["Tg"]
                xk = [src[:, 0, base:base + 2 * Tg:2],
                      src[:, 1, base:base + 2 * Tg:2],
                      src[:, 0, base + 1:base + 2 * Tg:2],
                      src[:, 1, base + 1:base + 2 * Tg:2]]
                return xk

            def dst_for(u):
                if u["kind"] == "l1":
                    return sA
                return sA if u["lev"] % 2 == 1 else sB

            def out_aps(u):
                if u["kind"] == "s0":
                    rt = ustate[s0_uid[(u["rg"], 0)]]["ring"]
                    lo = u["sub"] * u["Tg"]
                    return [rt[:, pt, lo:lo + u["Tg"]] for pt in range(DPT)]
                dst = dst_for(u)
                lo = u["base_out"]
                return [dst[:, pt, lo:lo + u["Tg"]] for pt in range(DPT)]

            # =========================================================
            # M phase
            # =========================================================
            def _combine(u, xk, par, gt, z, Tg):
                """z = (l+r+par) + t1*l + t2*r + t3*par"""
                lr = xk
                sacc = sb.tile([P, DPT, G], F32, tag="sacc",
                               name="sacc")[:, :, :Tg]
                m2p = sb.tile([P, DPT, G], F32, tag="m2p",
                              name="m2p")[:, :, :Tg]
                m3p = sb.tile([P, DPT, G], F32, tag="m3p",
                              name="m3p")[:, :, :Tg]
                for pt in range(DPT):
                    # s-chain on DVE (inputs available early)
                    nc.vector.tensor_add(sacc[:, pt, :], lr[pt], lr[2 + pt])
                    nc.vector.tensor_add(sacc[:, pt, :], sacc[:, pt, :],
                                         par[:, pt, :])
                    # gate products: t1*l on DVE, t2*r / t3*par on Pool
                    nc.gpsimd.tensor_mul(m2p[:, pt, :], gt[:, 2 + pt, :],
                                         lr[2 + pt])
                    nc.gpsimd.tensor_mul(m3p[:, pt, :], gt[:, 4 + pt, :],
                                         par[:, pt, :])
                    nc.vector.tensor_mul(z[:, pt, :], gt[:, pt, :], lr[pt])
                    nc.vector.tensor_add(z[:, pt, :], z[:, pt, :],
                                         sacc[:, pt, :])
                    nc.vector.tensor_add(z[:, pt, :], z[:, pt, :],
                                         m2p[:, pt, :])
                    nc.vector.tensor_add(z[:, pt, :], z[:, pt, :],
                                         m3p[:, pt, :])
                ustate[u["_id"]]["z"] = z

            def m_cell_pair(us):
                """Fused M for a pair of Tg=512 units: each weight tile is
                loaded once and used by both units' matmuls back-to-back."""
                Tg = us[0]["Tg"]
                xks = [in_aps(u) for u in us]
                hs = [hsb.tile([P, 8, G], BF16, tag="h",
                               name=f"h{j}")[:, :, :Tg]
                      for j in range(2)]
                for m in range(8):
                    phs = [ps.tile([P, 1, G], F32, tag="mm", name="ph",
                                   bufs=4)[:, 0, :Tg] for _ in range(2)]
                    for k in range(4):
                        for j in range(2):
                            nc.tensor.matmul(
                                phs[j], lhsT=w1s[:, k, m * P:(m + 1) * P],
                                rhs=xks[j][k], start=(k == 0), stop=(k == 3))
                    for j in range(2):
                        nc.scalar.activation(out=hs[j][:, m, :], in_=phs[j],
                                             func=AF.Gelu,
                                             bias=b1cs[:, m:m + 1])
                pars = [sb.tile([P, 2, G], BF16, tag="par",
                                name=f"par{j}")[:, :, :Tg] for j in range(2)]
                gts = [sb.tile([P, 6, G], BF16, tag="gt",
                               name=f"gt{j}")[:, :, :Tg] for j in range(2)]
                zs = [sb3.tile([P, DPT, G], F32R, tag="z", bufs=5,
                               name=f"z{j}")[:, :, :Tg] for j in range(2)]
                for m2 in (6, 7, 0, 1, 2, 3, 4, 5):   # parent first
                    pcs = [ps.tile([P, 1, G], F32, tag="mm", name="pc",
                                   bufs=4)[:, 0, :Tg] for _ in range(2)]
                    for k in range(8):
                        for j in range(2):
                            nc.tensor.matmul(
                                pcs[j], lhsT=w2s[:, k, m2 * P:(m2 + 1) * P],
                                rhs=hs[j][:, k, :], start=(k == 0),
                                stop=(k == 7))
                    for j in range(2):
                        if m2 >= 6:
                            nc.scalar.activation(
                                out=pars[j][:, m2 - 6, :], in_=pcs[j],
                                func=AF.Identity, bias=b2cs[:, m2:m2 + 1])
                        else:
                            nc.scalar.activation(
                                out=gts[j][:, m2, :], in_=pcs[j],
                                func=AF.Tanh, scale=0.5,
                                bias=b2hs[:, m2:m2 + 1])
                for j, u in enumerate(us):
                    _combine(u, xks[j], pars[j], gts[j], zs[j], Tg)

            def m_cell(u):
                Tg = u["Tg"]
                xk = in_aps(u)
                mpb = max(1, min(8, G // Tg))      # m-tiles per PSUM bank
                small = mpb > 1
                # ---- mm1 + gelu -> h ----
                h = hsb.tile([P, 8, G], BF16, tag="h", name="h")[:, :, :Tg]
                for b0 in range(0, 8, mpb):
                    nb = min(mpb, 8 - b0)
                    ph = ps.tile([P, mpb, G // mpb], F32, tag="mm",
                                 name="ph", bufs=4)[:, :nb, :Tg]
                    for mi in range(nb):
                        m = b0 + mi
                        for k in range(4):
                            nc.tensor.matmul(
                                ph[:, mi, :], lhsT=w1s[:, k, m * P:(m + 1) * P],
                                rhs=xk[k], start=(k == 0),
                                stop=(k == 3 and not small))
                        if small:
                            nc.tensor.matmul(
                                ph[:, mi, :], lhsT=b1rs[m],
                                rhs=onesr[:, :Tg], start=False, stop=True)
                    if small:
                        nc.scalar.activation(out=h[:, b0:b0 + nb, :], in_=ph,
                                             func=AF.Gelu)
                    else:
                        nc.scalar.activation(out=h[:, b0, :], in_=ph[:, 0, :],
                                             func=AF.Gelu,
                                             bias=b1cs[:, b0:b0 + 1])
                # ---- mm2 -> par (ident) + tanh gates ----
                par = sb.tile([P, 2, G], BF16, tag="par", name="par")[:, :, :Tg]
                gt = sb.tile([P, 6, G], BF16, tag="gt", name="gt")[:, :, :Tg]
                z = sb3.tile([P, DPT, G], F32R, tag="z", bufs=5,
                             name="z")[:, :, :Tg]
                m2_order = [6, 7, 0, 1, 2, 3, 4, 5]   # parent first
                bi = 0
                while bi < 8:
                    nb = min(mpb, 8 - bi)
                    ms_arr = m2_order[bi:bi + nb]
                    pc = ps.tile([P, mpb, G // mpb], F32, tag="mm",
                                 name="pc", bufs=4)[:, :nb, :Tg]
                    for mi, m2 in enumerate(ms_arr):
                        for k in range(8):
                            nc.tensor.matmul(
                                pc[:, mi, :],
                                lhsT=w2s[:, k, m2 * P:(m2 + 1) * P],
                                rhs=h[:, k, :], start=(k == 0),
                                stop=(k == 7 and not small))
                        if small:
                            nc.tensor.matmul(
                                pc[:, mi, :], lhsT=b2rs[m2],
                                rhs=onesr[:, :Tg], start=False, stop=True)
                    bi += nb
                    # consume this bank with ACT immediately
                    if small:
                        # contiguous runs of parent (6,7) vs gates (0..5)
                        st_ = 0
                        while st_ < len(ms_arr):
                            en = st_
                            is_par = ms_arr[st_] >= 6
                            while en + 1 < len(ms_arr) and \
                                    (ms_arr[en + 1] >= 6) == is_par:
                                en += 1
                            n_ = en - st_ + 1
                            if is_par:
                                nc.scalar.activation(
                                    out=par[:, ms_arr[st_] - 6:
                                            ms_arr[st_] - 6 + n_, :],
                                    in_=pc[:, st_:en + 1, :],
                                    func=AF.Identity)
                            else:
                                nc.scalar.activation(
                                    out=gt[:, ms_arr[st_]:
                                           ms_arr[st_] + n_, :],
                                    in_=pc[:, st_:en + 1, :],
                                    func=AF.Tanh, scale=0.5)
                            st_ = en + 1
                    else:
                        m2 = ms_arr[0]
                        if m2 >= 6:
                            nc.scalar.activation(
                                out=par[:, m2 - 6, :], in_=pc[:, 0, :],
                                func=AF.Identity, bias=b2cs[:, m2:m2 + 1])
                        else:
                            nc.scalar.activation(
                                out=gt[:, m2, :], in_=pc[:, 0, :],
                                func=AF.Tanh, scale=0.5,
                                bias=b2hs[:, m2:m2 + 1])
                _combine(u, xk, par, gt, z, Tg)

            def m_stage0(u):
                Tg = u["Tg"]
                rg, subi = u["rg"], u["sub"]
                if subi == 0:
                    rt = ring_p.tile([P, DPT, TR], BF16, tag="ring",
                                     name="ring")
                    ustate[u["_id"]]["ring"] = rt
                    for s_ in range(1, nsub):
                        ustate[s0_uid[(rg, s_)]]["ring"] = rt
                gi = rg * nsub + subi
                xr = x_d.rearrange("(a s p) d -> a p s d", p=P,
                                   s=max(sub_t // P, 1))
                itm = sb3.tile([P, max(sub_t // P, 1), D], F32R, tag="itm",
                               name="itm", bufs=2)
                nc.sync.dma_start(out=itm, in_=xr[gi])
                x0 = sb.tile([P, DPT, 512], F32R, tag="x0",
                             name="x0", bufs=1)[:, :, :Tg]
                for pt in range(DPT):
                    xtp = ps.tile([P, 512], F32R, tag="bc",
                                  name="xtp", bufs=2)[:, :Tg]
                    for s_ in range(max(sub_t // P, 1)):
                        nc.tensor.transpose(
                            xtp[:, s_ * P:(s_ + 1) * P],
                            itm[:, s_, pt * P:(pt + 1) * P], eyes)
                    nc.scalar.copy(out=x0[:, pt, :], in_=xtp)
                z = sb3.tile([P, DPT, G], F32R, tag="z", bufs=5,
                             name="z")[:, :, :Tg]
                for pt in range(DPT):
                    pw = ps.tile([P, 1, G], F32, tag="mm",
                                 name="pw", bufs=4)[:, 0, :Tg]
                    for k in range(DPT):
                        nc.tensor.matmul(pw, lhsT=wws[:, k, pt * P:(pt + 1) * P],
                                         rhs=x0[:, k, :],
                                         start=(k == 0), stop=(k == DPT - 1))
                    nc.scalar.activation(out=z[:, pt, :], in_=pw,
                                         func=AF.Identity,
                                         bias=bws[:, pt:pt + 1])
                ustate[u["_id"]]["z"] = z

            # =========================================================
            # S phase: stats matmuls + rows chain through rvar
            # =========================================================
            def s_phase(u):
                Tg = u["Tg"]
                z = ustate[u["_id"]]["z"]
                zsq = sb.tile([P, DPT, G], F32R, tag="zsq",
                              name="zsq")[:, :, :Tg]
                nc.scalar.activation(out=zsq, in_=z, func=AF.Square)
                st = psb.tile([1, G], F32, tag="st", name="st", bufs=1)[:, :Tg]
                sq = psb.tile([1, G], F32, tag="sq", name="sq", bufs=1)[:, :Tg]
                for pt in range(DPT):
                    nc.tensor.matmul(st, lhsT=oness, rhs=z[:, pt, :],
                                     start=(pt == 0), stop=(pt == DPT - 1))
                for pt in range(DPT):
                    nc.tensor.matmul(sq, lhsT=oness, rhs=zsq[:, pt, :],
                                     start=(pt == 0), stop=(pt == DPT - 1))
                mu = rows.tile([1, G], F32, tag="mu", name="mu", bufs=4)[:, :Tg]
                nc.vector.tensor_scalar_mul(mu, st, 1.0 / D)
                ex2 = rows.tile([1, G], F32, tag="ex2", name="ex2",
                                bufs=2)[:, :Tg]
                nc.vector.tensor_scalar_mul(ex2, sq, 1.0 / D)
                mq = rows.tile([1, G], F32, tag="mq", name="mq",
                               bufs=2)[:, :Tg]
                nc.vector.tensor_mul(mq, mu, mu)
                b = batch_of[u["_id"]]
                slot = batches[b].index(u["_id"])
                if "vb" not in ustate[batches[b][0]]:
                    vb = rows.tile([1, 2, G], F32, tag="vb", name="vb",
                                   bufs=2)
                    rsb = rows.tile([1, 2, G], F32R, tag="rsb", name="rsb",
                                    bufs=2)
                    for x in batches[b]:
                        ustate[x]["vb"] = vb
                        ustate[x]["rsb"] = rsb
                        ustate[x]["slot"] = batches[b].index(x)
                vb = ustate[u["_id"]]["vb"]
                nc.vector.scalar_tensor_tensor(
                    out=ex2, in0=ex2, scalar=EPS,
                    in1=mq, op0=ALU.add, op1=ALU.subtract)
                nc.vector.reciprocal(out=vb[:, slot, :Tg], in_=ex2)
                ustate[u["_id"]]["mu"] = mu

            # =========================================================
            # Q phase: batched sqrt (rsig = sqrt(1/var))
            # =========================================================
            def q_phase(bid):
                us = batches[bid]
                vb = ustate[us[0]]["vb"]
                rsb = ustate[us[0]]["rsb"]
                Tg = units[us[0]]["Tg"]
                if len(us) == 2:
                    nc.scalar.activation(out=rsb[:, :, :Tg],
                                         in_=vb[:, :, :Tg], func=AF.Sqrt)
                else:
                    nc.scalar.activation(out=rsb[:, 0, :Tg],
                                         in_=vb[:, 0, :Tg], func=AF.Sqrt)

            # =========================================================
            # A phase: ms, broadcast matmuls, apply -> state (bf16)
            # =========================================================
            def a_phase(u):
                Tg = u["Tg"]
                z = ustate[u["_id"]]["z"]
                mu = ustate[u["_id"]]["mu"]
                rsb = ustate[u["_id"]]["rsb"]
                slot = ustate[u["_id"]]["slot"]
                ln = 0 if u["kind"] == "s0" else 1
                rsig = rsb[:, slot, :Tg]
                ms = rows.tile([1, G], F32R, tag="ms", name="ms",
                               bufs=2)[:, :Tg]
                nc.vector.tensor_mul(ms, mu, rsig)
                oaps = out_aps(u)
                for pt in range(DPT):
                    b1p = ps.tile([P, G], F32, tag="bc", name="bc",
                                  bufs=2)[:, :Tg]
                    nc.tensor.matmul(b1p, lhsT=lnrs[2 * ln][:, pt * P:(pt + 1) * P],
                                     rhs=rsig, start=True, stop=True)
                    b2p = ps.tile([P, G], F32, tag="bc", name="bc",
                                  bufs=2)[:, :Tg]
                    nc.tensor.matmul(b2p, lhsT=lnrs[2 * ln + 1][:, pt * P:(pt + 1) * P],
                                     rhs=ms, start=True, stop=True)
                    t = sb.tile([P, G], F32, tag="tap", name="tap")[:, :Tg]
                    nc.vector.tensor_mul(t, z[:, pt, :], b1p)
                    nc.vector.scalar_tensor_tensor(
                        out=oaps[pt], in0=t, scalar=lnbs[:, ln, pt:pt + 1],
                        in1=b2p, op0=ALU.add, op1=ALU.add)

            # =========================================================
            # demand-driven scheduler
            # =========================================================
            PH_M, PH_S, PH_Q, PH_A = 0, 1, 2, 3
            phase = [None] * len(units)   # highest phase emitted
            q_done = set()
            q_tick = {}
            tick = [0]

            for i, u in enumerate(units):
                u["_id"] = i

            def emit_m(i):
                if phase[i] is not None:
                    return
                u = units[i]
                bid = batch_of[i]
                if u["kind"] == "s0":
                    m_stage0(u)
                    phase[i] = PH_M
                elif bid in m_fused:
                    pair = batches[bid]
                    for x in pair:
                        for d in units[x]["deps"]:
                            emit_a(d)
                    m_cell_pair([units[x] for x in pair])
                    for x in pair:
                        phase[x] = PH_M
                else:
                    m_cell(u)
                    phase[i] = PH_M

            def emit_s(i):
                if phase[i] is None:
                    emit_m(i)
                if phase[i] < PH_S:
                    s_phase(units[i])
                    phase[i] = PH_S

            def emit_q(bid):
                if bid in q_done:
                    return
                for x in batches[bid]:
                    emit_s(x)
                q_phase(bid)
                q_tick[bid] = tick[0]
                q_done.add(bid)
                for x in batches[bid]:
                    phase[x] = PH_Q

            def emit_a(i):
                if phase[i] is not None and phase[i] >= PH_A:
                    return
                emit_q(batch_of[i])
                a_phase(units[i])
                phase[i] = PH_A

            # main loop
            a_pending = []                # unit ids with Q done, A not
            for oi, i in enumerate(order):
                u = units[i]
                for d in u["deps"]:
                    emit_a(d)
                emit_m(i)
                tick[0] = oi
                # lagged S for the previously ordered unit
                if oi >= 1:
                    j = order[oi - 1]
                    if phase[j] < PH_S:
                        emit_s(j)
                        bid = batch_of[j]
                        if all(phase[x] is not None and phase[x] >= PH_S
                               for x in batches[bid]):
                            emit_q(bid)
                            a_pending.extend(batches[bid])
                # A for units whose Q is at least one tick old
                still = []
                for j in a_pending:
                    if phase[j] >= PH_A:
                        continue
                    if q_tick[batch_of[j]] < tick[0]:
                        emit_a(j)
                    else:
                        still.append(j)
                a_pending = still
            for i in range(len(units)):
                emit_a(i)

            # ---- emit output [R, D] (last level wrote 2 tokens) ----
            last_ids = lev_uids[-1]
            dst = dst_for(units[last_ids[0]])
            outt = sb.tile([R, D], F32, tag="outt", name="outt")
            for pt in range(DPT):
                otp = psb.tile([R, P], BF16, tag="st", name="otp", bufs=1)
                nc.tensor.transpose(otp, dst[:, pt, 0:R], eyebs)
                nc.vector.tensor_copy(out=outt[:, pt * P:(pt + 1) * P],
                                      in_=otp)
            nc.sync.dma_start(out=out_d, in_=outt)

        if iters == 1:
            body()
        else:
            with tc.For_i(0, iters, 1):
                body()

        for p_ in (psb, ps, rows, hsb, sb3, sb, ring_p, stp, cst):
            p_.release()

    nc.compile()
    return nc


def _prep_weights(w_word, b_word, w1, bias1, w2, bias2,
                  ln0_g, ln0_b, lnc_g, lnc_b):
    import ml_dtypes
    f = np.float32
    bf = ml_dtypes.bfloat16
    w1h = np.ascontiguousarray(
        w1.reshape(4, P, H).transpose(1, 0, 2), dtype=bf)
    w2h = np.ascontiguousarray(
        w2.reshape(8, P, H).transpose(1, 0, 2), dtype=bf)
    wwh = np.ascontiguousarray(w_word.reshape(2, P, D).transpose(1, 0, 2),
                               dtype=f)
    b1h = np.ascontiguousarray(bias1.reshape(8, P).T, dtype=f)
    b2h = np.ascontiguousarray(bias2.reshape(8, P).T, dtype=f)
    bwh = np.ascontiguousarray(b_word.reshape(2, P).T, dtype=f)
    lnrow = np.stack([ln0_g, -ln0_g, lnc_g, -lnc_g]).astype(f)    # [4, D]
    lnbcol = np.ascontiguousarray(
        np.stack([ln0_b, lnc_b]).reshape(2, 2, P).transpose(2, 0, 1), dtype=f)
    return dict(w1t=w1h, w2t=w2h, wwt=wwh, b1c=b1h, b2c=b2h,
                b2h=(b2h * 0.5).astype(f), bwc=bwh,
                b1r=np.ascontiguousarray(bias1.reshape(8, P), dtype=bf),
                b2r=np.ascontiguousarray(bias2.reshape(8, P), dtype=bf),
                lnrow=lnrow, lnbcol=lnbcol,
                onescol=np.ones((P, 1), f),
                onesrow=np.ones((1, G), bf),
                eye=np.eye(P, dtype=f),
                eyeb=np.eye(P, dtype=bf))


_NC_CACHE = {}


def _get_nc(S=S_FULL, iters=1, max_levels=None):
    key = (S, iters, max_levels)
    if key not in _NC_CACHE:
        _NC_CACHE[key] = _build(S, iters, max_levels)
    return _NC_CACHE[key]


def kernel(input, input_mask, w_word, b_word, w1, bias1, w2, bias2,
           ln0_g, ln0_b, lnc_g, lnc_b, _iters=1, _max_levels=None):
    inp = np.asarray(input, dtype=np.float32)
    shared = _prep_weights(
        np.asarray(w_word), np.asarray(b_word), np.asarray(w1),
        np.asarray(bias1), np.asarray(w2), np.asarray(bias2),
        np.asarray(ln0_g), np.asarray(ln0_b), np.asarray(lnc_g),
        np.asarray(lnc_b))
    S = inp.shape[1]
    nc = _get_nc(S, _iters, _max_levels)
    in_maps = []
    for c in range(N_CORES):
        m = dict(shared)
        m["x"] = np.ascontiguousarray(
            inp[c * R:(c + 1) * R].reshape(R * S, D))
        in_maps.append(m)
    res = run_bass_kernel_spmd(nc, in_maps, core_ids=list(range(N_CORES)))
    return np.concatenate([res.results[c]["out"] for c in range(N_CORES)],
                          axis=0)
